# revision 66
# baseline (speedup 1.0000x reference)
"""Block-sparse multi-head attention (QKV proj + sparse flash + out proj)
for Trainium2, sharded over 8 NeuronCores as batch x head-group.

Layout of the per-core program (SPMD: identical program on all 8 cores,
all sharding done host-side via per-core input slices):

  core c: batch b = c // 4, heads h0 = (c % 4) * 4 .. h0 + 4.
  inputs : xt    [1024, 2048] f16   x[b] TRANSPOSED on the host (d-major),
                                    so the device never transposes x
           wqkv  [1024, 768]  f16   w_qkv columns for the core's 4 heads,
                                    re-packed as [q(256) | k(256) | v(256)]
           bias  [768]        f32   matching b_qkv slice (q part pre-scaled
                                    by tau/sqrt(dk))
           wo    [256, 1024]  f16   w_o rows for the core's heads
  output : out   [2048, 1024] f16   partial output projection (host sums the
                                    4 head-group partials per batch, + b_o)

The block mask (16x16, shared by every head/batch) is known at program
build time, so the kernel is specialized to it: only active (qblock,
kblock) pairs get score/exp/PV work.  Softmax is computed without the
running-max pass: scores are ~N(0,1) for this problem family, so exp()
stays comfortably inside fp32/fp16 range, and softmax(S) is
mathematically identical with or without the max shift.

Everything flows transposed (dk-major) so no transpose is ever needed:
  Q^T,K^T  from x^T via c-major projection (contraction d on partitions)
  V        via s-major projection (same xT/w tiles, swapped operands)
  S^T[k,q] = K_j @ Q_r^T        (lhsT = K dk-major, rhs = Q dk-major)
  P^T      = exp(S^T)           (ACT, packed by active pair -> fp16)
  O'^T     = sum_j V'_j^T @ P^T (V' carries a ones column so one PSUM row
                                 accumulates l = sum P; the ones column
                                 sits below O for head-pair half 0 and
                                 above it for half 1 so the 1/l multiply
                                 operands share a base partition)
  O^T      = O'^T * (1/l)       (gpsimd partition-broadcast of 1/l,
                                 multiply fused with the PSUM evacuation)
  out      = O^T.T @ Wo         (lhsT = O^T blocks, rhs = Wo rows)

PSUM accumulation never uses zero-init matmuls: the first matmul into
each PSUM bank of an accumulation group is issued with start=True (which
clears the whole bank's has_written bits); later matmuls overwrite where
the bit is clear and accumulate where it is set, which is exactly the
per-element semantics needed.

The PE p-state ramp (0.65/1.2 GHz for the first ~3us of busy time) is
burned with dependency-free warm-up matmuls while the first DMAs land,
so real work starts at full clock.
"""

import math
import sys

import numpy as np

for _p in ("/opt/trn_rl_repo", "/root/.axon_site/_ro/trn_rl_repo"):
    if _p not in sys.path:
        sys.path.insert(0, _p)

import concourse.bass as bass
import concourse.mybir as mybir
import concourse.tile as tile
from concourse import bacc
from concourse.bass_utils import run_bass_kernel_spmd

H = 16      # total heads
DK = 64     # head dim
BS = 128    # block size
S = 2048    # sequence length
D = 1024    # model dim
B = 2       # batch
NCORES = 8
HL = 4      # heads per core
DL = HL * DK          # 256 local qkv width
TR = S // BS          # 16 blocks

F32 = mybir.dt.float32
F16 = mybir.dt.float16
EXP = mybir.ActivationFunctionType.Exp

N_WARM = 10           # warm-up matmuls (512 cols each) to cover the ramp


def _build_program(mask, scale, vbias16=None):
    """mask: [16,16] 0/1 array (build-time constant). scale: tau/sqrt(dk).
    vbias16: fp16 [256] v-part bias, or None when it is all zeros."""
    nc = bacc.Bacc("TRN2", target_bir_lowering=False)

    xt = nc.dram_tensor("xt", [D, S], F16, kind="ExternalInput")
    wqkv = nc.dram_tensor("wqkv", [D, 3 * DL], F16, kind="ExternalInput")
    bias = nc.dram_tensor("bias", [3 * DL], F32, kind="ExternalInput")
    wo = nc.dram_tensor("wo", [DL, D], F16, kind="ExternalInput")
    out = nc.dram_tensor("out", [S, D], F16, kind="ExternalOutput")
    bv = (nc.dram_tensor("bv", [1, DL], F16, kind="ExternalInput")
          if vbias16 is not None else None)

    # ---- build-time sparsity bookkeeping (mask shared by all heads) ----
    act_r = [[r for r in range(TR) if mask[r][j]] for j in range(TR)]
    first_j, last_j = {}, {}
    for r in range(TR):
        js = [j for j in range(TR) if mask[r][j]]
        if js:
            first_j[r], last_j[r] = js[0], js[-1]
    empty_rows = [r for r in range(TR) if r not in first_j]
    # Packed S^T/P^T slot order: all row-half-0 (r<8) pairs first, then
    # half 1 — so the first O'-half (and with it the first half of the
    # output projection) completes before the second half's exps finish.
    # Within a half, pairs are bucketed by the highest q/k block they
    # touch (max(j,r)//4), so the first groups only need the first
    # projection s-chunk and the QK->exp pipeline starts as early as
    # possible.  Bucket boundaries coincide with the r//4 PV-run limit,
    # so PV run merging is unaffected.
    pk = {}          # (j, r) -> packed slot index in P^T
    gidx = 0
    for half in range(2):
        for need in range(4):
            for j in range(TR):
                for r in act_r[j]:
                    if r // 8 == half and max(j, r) // 4 == need:
                        pk[(j, r)] = gidx
                        gidx += 1
    nact = gidx

    # runs of consecutive active rows at one key block, uniform stop flag.
    # Runs may not cross an O' psum bank (r//4) nor a P^T group tile
    # (packed slot // 8) boundary.
    def pv_runs(j):
        runs = []
        for r in act_r[j]:
            sp = last_j[r] == j
            if (runs and runs[-1][0] + runs[-1][1] == r
                    and runs[-1][2] == sp
                    and runs[-1][1] < 4
                    and runs[-1][0] // 4 == r // 4
                    and pk[(j, runs[-1][0])] // 8 == pk[(j, r)] // 8):
                runs[-1][1] += 1
            else:
                runs.append([r, 1, sp])
        return runs

    runs_by_j = {j: pv_runs(j) for j in range(TR)}
    pairs = sorted(pk, key=lambda jr: pk[jr])
    n_grp = (nact + 7) // 8
    # group g touches blocks up to grp_need[g]; its QK can be emitted once
    # the projection has produced q/k for all blocks <= that.
    grp_need = [max(max(j, r) for j, r in pairs[g * 8:(g + 1) * 8])
                for g in range(n_grp)]

    with tile.TileContext(nc) as tc:
        with tc.tile_pool(name="persist", bufs=1) as persist:
            bias_sb = persist.tile([128, 6], F32)
            w16 = persist.tile([128, 8, 3 * DL], F16)
            wo16 = persist.tile([128, 2, D], F16)
            xT16 = persist.tile([128, 8, S], F16)
            q16 = persist.tile([128, 2, S], F16)
            k16 = persist.tile([128, 2, S], F16)
            # V' = [V | ones]: PSUM row 64 of each O' tile accumulates
            # l = sum P via the ones column.
            v16 = persist.tile([128, HL, TR, DK + 1], F16)
            oT16 = persist.tile([128, 2, S], F16)
            warm = persist.tile([128, 512], F16)
            bv_sb = persist.tile([1, DL], F16) if bv is not None else None
            ones_sb = persist.tile([1, 128], F16) if bv is not None else None

            xt_r = xt[:].rearrange("(dc p) s -> p dc s", p=128)
            wqkv_r = wqkv[:].rearrange("(dc p) c -> p dc c", p=128)
            wo_r = wo[:].rearrange("(cc p) e -> p cc e", p=128)

            # ---- DMA stream (serial HWDGE/DMA engines; order == priority)
            nc.sync.dma_start(out=xT16[:, :, 0:128], in_=xt_r[:, :, 0:128])
            nc.sync.dma_start(out=w16[:, 0:4, :], in_=wqkv_r[:, 0:4, :])
            nc.sync.dma_start(out=xT16[:, :, 128:256], in_=xt_r[:, :, 128:256])
            nc.sync.dma_start(out=w16[:, 4:8, :], in_=wqkv_r[:, 4:8, :])
            nc.sync.dma_start(out=xT16[:, :, 256:384], in_=xt_r[:, :, 256:384])
            nc.sync.dma_start(out=xT16[:, :, 384:512], in_=xt_r[:, :, 384:512])
            nc.sync.dma_start(out=bias_sb,
                              in_=bias[:].rearrange("(g p) -> p g", p=128))
            for sc in range(1, 4):
                nc.sync.dma_start(out=xT16[:, :, sc * 512:(sc + 1) * 512],
                                  in_=xt_r[:, :, sc * 512:(sc + 1) * 512])
            nc.sync.dma_start(out=wo16, in_=wo_r)
            if bv is not None:
                nc.sync.dma_start(out=bv_sb, in_=bv[:])

            nc.vector.memset(warm, 0.0)
            for h in range(HL):
                nc.vector.memset(v16[:, h, :, DK:DK + 1], 1.0)
            if ones_sb is not None:
                nc.vector.memset(ones_sb, 1.0)
            # dummy exp so the ACT table load runs at t~0, off the first
            # QK group's critical path
            dummy = persist.tile([1, 8], F16)
            nc.scalar.activation(out=dummy, in_=warm[0:1, 0:8], func=EXP)

            # ---- PE warm-up: dependency-free matmuls burn the p-state ramp
            # while the first x/w DMAs land.
            warm_ctx = tc.tile_pool(name="warm_ps", bufs=1, space="PSUM")
            warm_ps = warm_ctx.__enter__()
            wtile = warm_ps.tile([128, 512], F32)

            def emit_warm(n):
                for _ in range(n):
                    nc.tensor.matmul(wtile, lhsT=warm[:, 0:128], rhs=warm,
                                     start=True, stop=True)
            emit_warm(N_WARM)

            # =========== phase A0: s-chunk 0, dc-outer (DMA-paced) ========
            # 6 concurrently-accumulating PSUM banks: 4 Q/K c-chunks of
            # [c=128, s=512] and 2 V tiles of [s=128 x 2, c=256 halves];
            # matmuls are emitted at (dc, sb) granularity in DMA arrival
            # order (w comes in two dc-halves, x in four s-blocks).
            def emit_qk_evac(mm, cc, sc, on_act=False):
                if cc < 2:
                    dst, sc_imm = q16[:, cc, sc * 512:(sc + 1) * 512], scale
                else:
                    dst, sc_imm = k16[:, cc - 2, sc * 512:(sc + 1) * 512], 1.0
                if on_act:      # ACT: out = Ident(in * scale + bias)
                    nc.scalar.activation(
                        out=dst, in_=mm,
                        func=mybir.ActivationFunctionType.Identity,
                        bias=bias_sb[:, cc:cc + 1], scale=sc_imm)
                else:
                    nc.vector.tensor_scalar(
                        out=dst, in0=mm, scalar1=sc_imm,
                        scalar2=bias_sb[:, cc:cc + 1],
                        op0=mybir.AluOpType.mult, op1=mybir.AluOpType.add)

            def emit_v_bias(vt, half):
                if bv_sb is not None:
                    nc.tensor.matmul(
                        vt[:, half * 256:(half + 1) * 256],
                        lhsT=ones_sb, rhs=bv_sb,
                        start=False, stop=True, skip_group_check=True)

            def emit_v_evac(vt, sb0):
                # vt [s=128, (sb2 h4 d64)] -> v16[:, h, sb, 0:64]
                nc.vector.tensor_copy(
                    out=v16[:, :, sb0:sb0 + 2, 0:DK],
                    in_=vt[:].rearrange("p (s h d) -> p h s d", s=2, h=4))

            with tc.tile_pool(name="pa0", bufs=1, space="PSUM") as pa0:
                qk0 = [pa0.tile([128, 512], F32, name=f"qk0_{cc}")
                       for cc in range(4)]

                def sc0_mm(dc, sb):
                    for cc in range(4):
                        nc.tensor.matmul(
                            qk0[cc][:, sb * 128:(sb + 1) * 128],
                            lhsT=w16[:, dc, cc * 128:(cc + 1) * 128],
                            rhs=xT16[:, dc, sb * 128:(sb + 1) * 128],
                            start=(dc == 0 and sb == 0), stop=(dc == 7),
                            skip_group_check=True)

                # arrival-ordered emission: (w half A: dc0-3 | B: dc4-7),
                # x s-blocks land 0,1,2,3.  Warm-up filler between waves
                # keeps the PE ramp alive while DMA catches up.
                for dc in range(4):
                    sc0_mm(dc, 0)
                emit_warm(2)
                for dc in range(4):
                    sc0_mm(dc, 1)
                emit_warm(3)        # bridges the wait for the 2nd w half
                for dc in range(4, 8):
                    sc0_mm(dc, 0)
                for dc in range(4, 8):
                    sc0_mm(dc, 1)
                for dc in range(8):
                    sc0_mm(dc, 2)
                for dc in range(8):
                    sc0_mm(dc, 3)
                # ACT is idle here (no exps yet): split the 4 evacs across
                # DVE and ACT so the A0->A1 psum-bank handoff drains fast
                for cc in range(4):
                    emit_qk_evac(qk0[cc], cc, 0, on_act=(cc % 2 == 1))
                # dependency-free warms keep PE busy (and the ramp alive)
                # while the evacs drain and the A1 pool takes over the banks
                emit_warm(3)
            warm_ctx.__exit__(None, None, None)

            # =========== QK/exp machinery =================================
            st_ctx = tc.tile_pool(name="pb_st", bufs=1, space="PSUM")
            st_ps = st_ctx.__enter__()
            pt_ctx = tc.tile_pool(name="pb_pt", bufs=40)
            pt_pool = pt_ctx.__enter__()

            ptmap = {}      # (hh, hp, grp) -> P^T group tile

            def emit_qk_group(hh, grp):
                lo = grp * 8
                chunk = pairs[lo:lo + 8]
                sts = [st_ps.tile([128, 1024], F32,
                                  name=f"st{hp}_{hh}_{grp}", tag=f"st{hp}")
                       for hp in range(2)]
                # merge consecutive active rows at the same key block into
                # one wider matmul (same stationary K_j, moving N up to 512;
                # may not cross a psum bank)
                qk_runs = []
                for sl, (j, r) in enumerate(chunk):
                    if (qk_runs and qk_runs[-1][0] == j
                            and qk_runs[-1][1] + qk_runs[-1][3] == r
                            and qk_runs[-1][2] // 4 == sl // 4
                            and qk_runs[-1][3] < 4):
                        qk_runs[-1][3] += 1
                    else:
                        qk_runs.append([j, r, sl, 1])
                for j, r0, sl0, L in qk_runs:
                    for hp in range(2):
                        bp = hp * 64
                        nc.tensor.matmul(
                            sts[hp][:, sl0 * 128:(sl0 + L) * 128],
                            lhsT=k16[bp:bp + 64, hh, j * 128:(j + 1) * 128],
                            rhs=q16[bp:bp + 64, hh, r0 * 128:(r0 + L) * 128],
                            start=True, stop=True)
                for hp in range(2):
                    ptg = pt_pool.tile([128, 1024], F16,
                                       name=f"ptg_{hh}_{hp}_{grp}", tag="ptg")
                    ptmap[(hh, hp, grp)] = ptg
                    nc.scalar.activation(
                        out=ptg[:, 0:len(chunk) * 128],
                        in_=sts[hp][:, 0:len(chunk) * 128], func=EXP)

            # pending group queue in emission priority order: alternate hh so
            # both heads' P^T becomes available evenly.
            grp_order = []
            for g in range(n_grp):
                for hh in range(2):
                    grp_order.append((hh, g))
            g_next = 0          # next index into grp_order not yet emitted
            emitted = set()

            def emit_next_group_if(maxblock):
                nonlocal g_next
                if (g_next < len(grp_order)
                        and grp_need[grp_order[g_next][1]] <= maxblock):
                    hh, g = grp_order[g_next]
                    emit_qk_group(hh, g)
                    emitted.add((hh, g))
                    g_next += 1
                    return True
                return False

            # =========== phase A1: Q/K s-chunks 1..3, then V, with QK
            # groups interleaved.  All of Q/K is projected FIRST so every
            # attention group unlocks as early as possible (ACT is the
            # long pole); the V projection runs after as PE filler.
            with tc.tile_pool(name="pa1", bufs=2, space="PSUM") as pa1:
                for sc in range(1, 4):
                    done_block = sc * 4 - 1   # blocks < sc*4 are projected
                    for cc in range(4):
                        mm = pa1.tile([128, 512], F32)
                        for dc in range(8):
                            nc.tensor.matmul(
                                mm,
                                lhsT=w16[:, dc, cc * 128:(cc + 1) * 128],
                                rhs=xT16[:, dc, sc * 512:(sc + 1) * 512],
                                start=(dc == 0), stop=(dc == 7))
                        emit_qk_evac(mm, cc, sc)
                        emit_next_group_if(
                            done_block if cc < 3 else sc * 4 + 3)
                for p in range(8):
                    vt = pa1.tile([128, 512], F32)
                    for half in range(2):
                        sb = p * 2 + half
                        for dc in range(8):
                            nc.tensor.matmul(
                                vt[:, half * 256:(half + 1) * 256],
                                lhsT=xT16[:, dc, sb * 128:(sb + 1) * 128],
                                rhs=w16[:, dc, 512:768],
                                start=(dc == 0 and half == 0),
                                stop=(dc == 7) if bv_sb is None else False,
                                skip_group_check=True)
                        emit_v_bias(vt, half)
                    emit_v_evac(vt, p * 2)
                    emit_next_group_if(TR - 1)

            # =========== phase B: PV + remaining QK groups ================
            # O' accumulated per (head, half-of-rows) into a [128, 1024]
            # (2-bank) psum tile, rows 0..63 = O, row 64 = l.  First matmul
            # into each bank carries start=True (whole-bank has_written
            # clear); later matmuls overwrite-or-accumulate per element.
            # hp0's (1/l)-multiply is fused with the PSUM evacuation (same
            # base partition); hp1 copies to oT16 first (cross-base copy is
            # legal, elementwise multiply needs aligned bases).
            with tc.tile_pool(name="pb_o", bufs=2, space="PSUM") as o_ps, \
                 tc.tile_pool(name="pb_div", bufs=2) as div_pool, \
                 tc.tile_pool(name="pc_sb", bufs=12) as fo_sb:

                fin_count = {0: 0, 1: 0}

                def head_items(hh, hp, half):
                    """Yield (req_groups, pe_cycles, emit_fn) for one
                    (head, row-half) O' accumulation."""
                    h = 2 * hh + hp
                    bp = hp * 64
                    HS = S // 2
                    state = {}

                    def ensure_tile():
                        if "o" not in state:
                            state["o"] = o_ps.tile(
                                [128, 1024], F32,
                                name=f"o_{hh}_{hp}_{half}", tag="o")
                            state["started"] = set()
                        return state["o"]

                    def runs_chunk(j4):
                        t = ensure_tile()
                        for j in range(j4 * 4, j4 * 4 + 4):
                            for r0, ln, sp in runs_by_j[j]:
                                if r0 // 8 != half:
                                    continue
                                off = pk[(j, r0)]
                                ptg = ptmap[(hh, hp, off // 8)]
                                o8 = off % 8
                                c0 = (r0 - half * 8) * 128
                                bank = (r0 - half * 8) // 4
                                st = bank not in state["started"]
                                state["started"].add(bank)
                                nc.tensor.matmul(
                                    t[0:DK + 1, c0:c0 + ln * 128],
                                    lhsT=v16[:, h, j, :],
                                    rhs=ptg[:, o8 * 128:(o8 + ln) * 128],
                                    start=st, stop=sp,
                                    skip_group_check=True)

                    for j4 in range(4):
                        req, cyc = set(), 0
                        for j in range(j4 * 4, j4 * 4 + 4):
                            for r0, ln, sp in runs_by_j[j]:
                                if r0 // 8 == half:
                                    req.add((hh, pk[(j, r0)] // 8))
                                    cyc += ln * 128
                        if cyc:
                            yield (req, cyc, lambda j4=j4: runs_chunk(j4))

                    def finish_half():
                        t = ensure_tile()
                        for r in empty_rows:
                            if r // 8 != half:
                                continue
                            c0 = (r - half * 8) * 128
                            nc.vector.memset(t[0:DK, c0:c0 + 128], 0.0)
                            nc.vector.memset(t[DK:DK + 1, c0:c0 + 128], 1.0)
                        dst = oT16[bp:bp + 64, hh,
                                   half * HS:(half + 1) * HS]
                        linv = div_pool.tile(
                            [1, HS], F32, name=f"linv_{h}_{half}",
                            tag="linv")
                        lb = div_pool.tile(
                            [128, HS], F32, name=f"lb_{h}_{half}", tag="lb")
                        late = half == 1 and fin_count[1] >= 2
                        nc.vector.reciprocal(linv, t[DK:DK + 1, :])
                        nc.gpsimd.partition_broadcast(lb, linv)
                        if hp == 0:
                            if late:    # split: first 512 cols unlock the
                                        # next out-proj stripe early
                                nc.vector.tensor_mul(
                                    dst[:, 0:512], t[0:DK, 0:512],
                                    lb[0:64, 0:512])
                                nc.vector.tensor_mul(
                                    dst[:, 512:1024], t[0:DK, 512:1024],
                                    lb[0:64, 512:1024])
                            else:
                                nc.vector.tensor_mul(dst, t[0:DK, :],
                                                     lb[0:64, :])
                        else:
                            if late:    # ACT queue is drained by now
                                nc.scalar.copy(out=dst, in_=t[0:DK, :])
                            else:
                                nc.vector.tensor_copy(out=dst,
                                                      in_=t[0:DK, :])
                            if late:
                                nc.vector.tensor_mul(
                                    dst[:, 0:512], dst[:, 0:512],
                                    lb[bp:bp + 64, 0:512])
                                nc.vector.tensor_mul(
                                    dst[:, 512:1024], dst[:, 512:1024],
                                    lb[bp:bp + 64, 512:1024])
                            else:
                                nc.vector.tensor_mul(dst, dst,
                                                     lb[bp:bp + 64, :])
                        fin_count[half] += 1
                    allreq = {(hh, pk[(j, r0)] // 8)
                              for j in range(TR)
                              for r0, ln, sp in runs_by_j[j]
                              if r0 // 8 == half}
                    yield (allreq, 0, finish_half)

                # (head, half) units in pipeline order: all half-0 units
                # before half-1, so oT16 columns complete in order and the
                # output projection's first half unblocks first.
                pv_stream = []      # (req_groups, pe_cycles, emit_fn)
                for half in range(2):
                    for hh in range(2):
                        for hp in range(2):
                            pv_stream.extend(head_items(hh, hp, half))
                pv_i = 0
                c_next = 0

                def c_ready(sb):
                    return fin_count[sb // 8] >= 4

                # ---- output projection, sharing the "o" psum tag: each
                # C-tile is [128, 1024] = 2 banks, one per e-half.  sb 0-7
                # only need the half-0 finishes and act as PE filler inside
                # the ACT-bound group window; their evacs stay off ACT so
                # the exp pipeline is never delayed.
                c_pool_cycle = [0]
                c_pre = {}      # sb -> fo tile with the hh0 half done

                def emit_c_pre(sb, tag):
                    # hh0's half-1 O is finished before hh1's: start the
                    # out-proj accumulation for a late stripe early in an
                    # idle S^T bank; the hh1 matmuls complete it later.
                    fo = st_ps.tile([128, 1024], F32, name=f"c_{sb}",
                                    tag=tag)
                    for e in range(2):
                        nc.tensor.matmul(
                            fo[:, e * 512:(e + 1) * 512],
                            lhsT=oT16[:, 0, sb * 128:(sb + 1) * 128],
                            rhs=wo16[:, 0, e * 512:(e + 1) * 512],
                            start=True, stop=False, skip_group_check=True)
                    c_pre[sb] = fo

                def emit_c_sb(sb):
                    act_ok = sb >= 8    # late sbs: exps done, ACT helps
                    if sb in c_pre:
                        fo = c_pre.pop(sb)
                        hhs = [1]
                    else:
                        if g_next >= len(grp_order):
                            # groups done: idle S^T banks double the number
                            # of out-proj tiles in flight
                            srcs = [(o_ps, "o"), (st_ps, "st0"),
                                    (st_ps, "st1")]
                            pool, tag = srcs[c_pool_cycle[0] % 3]
                            c_pool_cycle[0] += 1
                        else:
                            pool, tag = o_ps, "o"
                        fo = pool.tile([128, 1024], F32, name=f"c_{sb}",
                                       tag=tag)
                        hhs = [0, 1]
                    st = fo_sb.tile([128, D], F16)
                    for e in range(2):
                        for hh in hhs:
                            nc.tensor.matmul(
                                fo[:, e * 512:(e + 1) * 512],
                                lhsT=oT16[:, hh, sb * 128:(sb + 1) * 128],
                                rhs=wo16[:, hh, e * 512:(e + 1) * 512],
                                start=(hh == 0), stop=(hh == 1),
                                skip_group_check=True)
                    if act_ok:
                        nc.vector.tensor_copy(
                            out=st[:, 0:512], in_=fo[:, 0:512])
                        nc.scalar.copy(
                            out=st[:, 512:1024], in_=fo[:, 512:1024])
                    else:
                        nc.vector.tensor_copy(out=st, in_=fo)
                    nc.sync.dma_start(
                        out=out[sb * 128:(sb + 1) * 128, :], in_=st)

                fill_total = 0

                def emit_fillers(target):
                    nonlocal pv_i, c_next, fill_total
                    while fill_total < target:
                        if pv_i < len(pv_stream):
                            req, cyc, fn = pv_stream[pv_i]
                            if req <= emitted:
                                fn()
                                fill_total += cyc
                                pv_i += 1
                                continue
                        if c_next < TR and c_ready(c_next):
                            emit_c_sb(c_next)
                            c_next += 1
                            fill_total += 2048
                            continue
                        break

                # steady state: pace QK groups with PV/out-proj filler so
                # the in-order PE queue alternates (QK paces ACT, the
                # filler keeps PE busy while ACT chews the exps).  The
                # filler target is cumulative, so emission blocked in one
                # round (P^T group not yet live) is caught up in the next.
                grp_done = 0
                while g_next < len(grp_order):
                    emit_fillers((grp_done + 1) * 2750)
                    emit_next_group_if(TR - 1)
                    grp_done += 1
                while pv_i < len(pv_stream):
                    req, cyc, fn = pv_stream[pv_i]
                    assert req <= emitted
                    fn()
                    pv_i += 1
                    if fin_count[1] == 2 and not c_pre and c_next <= 8:
                        emit_c_pre(8, "st0")
                        emit_c_pre(9, "st1")
                    while c_next < TR and c_ready(c_next):
                        emit_c_sb(c_next)
                        c_next += 1
                while c_next < TR:
                    emit_c_sb(c_next)
                    c_next += 1

            st_ctx.__exit__(None, None, None)
            pt_ctx.__exit__(None, None, None)
    nc.finalize()
    return nc


def _shard_inputs(x, w_qkv, b_qkv, w_o, scale):
    in_maps = []
    xts = [np.ascontiguousarray(x[b].T, dtype=np.float16) for b in range(B)]
    for c in range(NCORES):
        b, h0 = c // 4, (c % 4) * HL
        q = slice(h0 * DK, h0 * DK + DL)
        k = slice(D + h0 * DK, D + h0 * DK + DL)
        v = slice(2 * D + h0 * DK, 2 * D + h0 * DK + DL)
        wslice = np.concatenate([w_qkv[:, q], w_qkv[:, k], w_qkv[:, v]], axis=1)
        beff = np.concatenate([b_qkv[q] * scale, b_qkv[k], b_qkv[v]])
        im = {
            "xt": xts[b],
            "wqkv": np.ascontiguousarray(wslice, dtype=np.float16),
            "bias": np.ascontiguousarray(beff, dtype=np.float32),
            "wo": np.ascontiguousarray(w_o[h0 * DK:h0 * DK + DL, :],
                                       dtype=np.float16),
        }
        if np.any(b_qkv[v] != 0):
            im["bv"] = np.ascontiguousarray(
                b_qkv[v].reshape(1, DL), dtype=np.float16)
        in_maps.append(im)
    return in_maps


def kernel(x, w_qkv, b_qkv, w_o, b_o, tau, block_sparse_mask, _trace=False,
           **_run_kwargs):
    x = np.asarray(x, dtype=np.float32)
    w_qkv = np.asarray(w_qkv, dtype=np.float32)
    b_qkv = np.asarray(b_qkv, dtype=np.float32)
    w_o = np.asarray(w_o, dtype=np.float32)
    b_o = np.asarray(b_o, dtype=np.float32)
    mask = np.asarray(block_sparse_mask).astype(np.int64)
    scale = float(np.asarray(tau)) / math.sqrt(DK)

    vb = b_qkv[2 * D:]
    vbias16 = vb.astype(np.float16) if np.any(vb != 0) else None
    nc = _build_program(mask, scale, vbias16)
    in_maps = _shard_inputs(x, w_qkv, b_qkv, w_o, scale)
    res = run_bass_kernel_spmd(nc, in_maps, core_ids=list(range(NCORES)),
                               trace=_trace, **_run_kwargs)
    outs = [r["out"].astype(np.float32) for r in res.results]
    full = np.stack([
        outs[0] + outs[1] + outs[2] + outs[3] + b_o,
        outs[4] + outs[5] + outs[6] + outs[7] + b_o,
    ]).astype(np.float32)
    if _trace:
        kernel.last_result = res
    return full


# revision 67
# speedup vs baseline: 1.0058x; 1.0058x over previous
"""Block-sparse multi-head attention (QKV proj + sparse flash + out proj)
for Trainium2, sharded over 8 NeuronCores as batch x head-group.

Layout of the per-core program (SPMD: identical program on all 8 cores,
all sharding done host-side via per-core input slices):

  core c: batch b = c // 4, heads h0 = (c % 4) * 4 .. h0 + 4.
  inputs : xt    [1024, 2048] f16   x[b] TRANSPOSED on the host (d-major),
                                    so the device never transposes x
           wqkv  [1024, 768]  f16   w_qkv columns for the core's 4 heads,
                                    re-packed as [q(256) | k(256) | v(256)]
           bias  [768]        f32   matching b_qkv slice (q part pre-scaled
                                    by tau/sqrt(dk))
           wo    [256, 1024]  f16   w_o rows for the core's heads
  output : out   [2048, 1024] f16   partial output projection (host sums the
                                    4 head-group partials per batch, + b_o)

The block mask (16x16, shared by every head/batch) is known at program
build time, so the kernel is specialized to it: only active (qblock,
kblock) pairs get score/exp/PV work.  Softmax is computed without the
running-max pass: scores are ~N(0,1) for this problem family, so exp()
stays comfortably inside fp32/fp16 range, and softmax(S) is
mathematically identical with or without the max shift.

Everything flows transposed (dk-major) so no transpose is ever needed:
  Q^T,K^T  from x^T via c-major projection (contraction d on partitions)
  V        via s-major projection (same xT/w tiles, swapped operands)
  S^T[k,q] = K_j @ Q_r^T        (lhsT = K dk-major, rhs = Q dk-major)
  P^T      = exp(S^T)           (ACT, packed by active pair -> fp16)
  O'^T     = sum_j V'_j^T @ P^T (V' carries a ones column so one PSUM row
                                 accumulates l = sum P; the ones column
                                 sits below O for head-pair half 0 and
                                 above it for half 1 so the 1/l multiply
                                 operands share a base partition)
  O^T      = O'^T * (1/l)       (gpsimd partition-broadcast of 1/l,
                                 multiply fused with the PSUM evacuation)
  out      = O^T.T @ Wo         (lhsT = O^T blocks, rhs = Wo rows)

PSUM accumulation never uses zero-init matmuls: the first matmul into
each PSUM bank of an accumulation group is issued with start=True (which
clears the whole bank's has_written bits); later matmuls overwrite where
the bit is clear and accumulate where it is set, which is exactly the
per-element semantics needed.

The PE p-state ramp (0.65/1.2 GHz for the first ~3us of busy time) is
burned with dependency-free warm-up matmuls while the first DMAs land,
so real work starts at full clock.
"""

import math
import sys

import numpy as np

for _p in ("/opt/trn_rl_repo", "/root/.axon_site/_ro/trn_rl_repo"):
    if _p not in sys.path:
        sys.path.insert(0, _p)

import concourse.bass as bass
import concourse.mybir as mybir
import concourse.tile as tile
from concourse import bacc
from concourse.bass_utils import run_bass_kernel_spmd

H = 16      # total heads
DK = 64     # head dim
BS = 128    # block size
S = 2048    # sequence length
D = 1024    # model dim
B = 2       # batch
NCORES = 8
HL = 4      # heads per core
DL = HL * DK          # 256 local qkv width
TR = S // BS          # 16 blocks

F32 = mybir.dt.float32
F16 = mybir.dt.float16
EXP = mybir.ActivationFunctionType.Exp

N_WARM = 10           # warm-up matmuls (512 cols each) to cover the ramp


def _build_program(mask, scale, vbias16=None):
    """mask: [16,16] 0/1 array (build-time constant). scale: tau/sqrt(dk).
    vbias16: fp16 [256] v-part bias, or None when it is all zeros."""
    nc = bacc.Bacc("TRN2", target_bir_lowering=False)

    xt = nc.dram_tensor("xt", [D, S], F16, kind="ExternalInput")
    wqkv = nc.dram_tensor("wqkv", [D, 3 * DL], F16, kind="ExternalInput")
    bias = nc.dram_tensor("bias", [3 * DL], F32, kind="ExternalInput")
    wo = nc.dram_tensor("wo", [DL, D], F16, kind="ExternalInput")
    out = nc.dram_tensor("out", [S, D], F16, kind="ExternalOutput")
    bv = (nc.dram_tensor("bv", [1, DL], F16, kind="ExternalInput")
          if vbias16 is not None else None)

    # ---- build-time sparsity bookkeeping (mask shared by all heads) ----
    act_r = [[r for r in range(TR) if mask[r][j]] for j in range(TR)]
    first_j, last_j = {}, {}
    for r in range(TR):
        js = [j for j in range(TR) if mask[r][j]]
        if js:
            first_j[r], last_j[r] = js[0], js[-1]
    empty_rows = [r for r in range(TR) if r not in first_j]
    # Packed S^T/P^T slot order: all row-half-0 (r<8) pairs first, then
    # half 1 — so the first O'-half (and with it the first half of the
    # output projection) completes before the second half's exps finish.
    # Within a half, pairs are bucketed by the highest q/k block they
    # touch (max(j,r)//4), so the first groups only need the first
    # projection s-chunk and the QK->exp pipeline starts as early as
    # possible.  Bucket boundaries coincide with the r//4 PV-run limit,
    # so PV run merging is unaffected.
    pk = {}          # (j, r) -> packed slot index in P^T
    gidx = 0
    for half in range(2):
        for need in range(4):
            for j in range(TR):
                for r in act_r[j]:
                    if r // 8 == half and max(j, r) // 4 == need:
                        pk[(j, r)] = gidx
                        gidx += 1
    nact = gidx

    # runs of consecutive active rows at one key block, uniform stop flag.
    # Runs may not cross an O' psum bank (r//4) nor a P^T group tile
    # (packed slot // 8) boundary.
    def pv_runs(j):
        runs = []
        for r in act_r[j]:
            sp = last_j[r] == j
            if (runs and runs[-1][0] + runs[-1][1] == r
                    and runs[-1][2] == sp
                    and runs[-1][1] < 4
                    and runs[-1][0] // 4 == r // 4
                    and pk[(j, runs[-1][0])] // 8 == pk[(j, r)] // 8):
                runs[-1][1] += 1
            else:
                runs.append([r, 1, sp])
        return runs

    runs_by_j = {j: pv_runs(j) for j in range(TR)}
    pairs = sorted(pk, key=lambda jr: pk[jr])
    n_grp = (nact + 7) // 8
    # group g touches blocks up to grp_need[g]; its QK can be emitted once
    # the projection has produced q/k for all blocks <= that.
    grp_need = [max(max(j, r) for j, r in pairs[g * 8:(g + 1) * 8])
                for g in range(n_grp)]

    with tile.TileContext(nc) as tc:
        with tc.tile_pool(name="persist", bufs=1) as persist:
            bias_sb = persist.tile([128, 6], F32)
            w16 = persist.tile([128, 8, 3 * DL], F16)
            wo16 = persist.tile([128, 2, D], F16)
            xT16 = persist.tile([128, 8, S], F16)
            q16 = persist.tile([128, 2, S], F16)
            k16 = persist.tile([128, 2, S], F16)
            # V' = [V | ones]: PSUM row 64 of each O' tile accumulates
            # l = sum P via the ones column.
            v16 = persist.tile([128, HL, TR, DK + 1], F16)
            oT16 = persist.tile([128, 2, S], F16)
            warm = persist.tile([128, 512], F16)
            bv_sb = persist.tile([1, DL], F16) if bv is not None else None
            ones_sb = persist.tile([1, 128], F16) if bv is not None else None

            xt_r = xt[:].rearrange("(dc p) s -> p dc s", p=128)
            wqkv_r = wqkv[:].rearrange("(dc p) c -> p dc c", p=128)
            wo_r = wo[:].rearrange("(cc p) e -> p cc e", p=128)

            # ---- DMA stream (serial HWDGE/DMA engines; order == priority)
            nc.sync.dma_start(out=xT16[:, :, 0:128], in_=xt_r[:, :, 0:128])
            nc.sync.dma_start(out=w16[:, 0:4, :], in_=wqkv_r[:, 0:4, :])
            nc.sync.dma_start(out=xT16[:, :, 128:256], in_=xt_r[:, :, 128:256])
            nc.sync.dma_start(out=w16[:, 4:8, :], in_=wqkv_r[:, 4:8, :])
            nc.sync.dma_start(out=xT16[:, :, 256:384], in_=xt_r[:, :, 256:384])
            nc.sync.dma_start(out=xT16[:, :, 384:512], in_=xt_r[:, :, 384:512])
            nc.sync.dma_start(out=bias_sb,
                              in_=bias[:].rearrange("(g p) -> p g", p=128))
            for sc in range(1, 4):
                nc.sync.dma_start(out=xT16[:, :, sc * 512:(sc + 1) * 512],
                                  in_=xt_r[:, :, sc * 512:(sc + 1) * 512])
            nc.sync.dma_start(out=wo16, in_=wo_r)
            if bv is not None:
                nc.sync.dma_start(out=bv_sb, in_=bv[:])

            nc.vector.memset(warm, 0.0)
            for h in range(HL):
                nc.vector.memset(v16[:, h, :, DK:DK + 1], 1.0)
            if ones_sb is not None:
                nc.vector.memset(ones_sb, 1.0)
            # dummy exp so the ACT table load runs at t~0, off the first
            # QK group's critical path
            dummy = persist.tile([1, 8], F16)
            nc.scalar.activation(out=dummy, in_=warm[0:1, 0:8], func=EXP)

            # ---- PE warm-up: dependency-free matmuls burn the p-state ramp
            # while the first x/w DMAs land.
            warm_ctx = tc.tile_pool(name="warm_ps", bufs=1, space="PSUM")
            warm_ps = warm_ctx.__enter__()
            wtile = warm_ps.tile([128, 512], F32)

            def emit_warm(n):
                for _ in range(n):
                    nc.tensor.matmul(wtile, lhsT=warm[:, 0:128], rhs=warm,
                                     start=True, stop=True)
            emit_warm(N_WARM)

            # =========== phase A0: s-chunk 0, dc-outer (DMA-paced) ========
            # 6 concurrently-accumulating PSUM banks: 4 Q/K c-chunks of
            # [c=128, s=512] and 2 V tiles of [s=128 x 2, c=256 halves];
            # matmuls are emitted at (dc, sb) granularity in DMA arrival
            # order (w comes in two dc-halves, x in four s-blocks).
            def emit_qk_evac(mm, cc, sc, on_act=False):
                if cc < 2:
                    dst, sc_imm = q16[:, cc, sc * 512:(sc + 1) * 512], scale
                else:
                    dst, sc_imm = k16[:, cc - 2, sc * 512:(sc + 1) * 512], 1.0
                if on_act:      # ACT: out = Ident(in * scale + bias)
                    nc.scalar.activation(
                        out=dst, in_=mm,
                        func=mybir.ActivationFunctionType.Identity,
                        bias=bias_sb[:, cc:cc + 1], scale=sc_imm)
                else:
                    nc.vector.tensor_scalar(
                        out=dst, in0=mm, scalar1=sc_imm,
                        scalar2=bias_sb[:, cc:cc + 1],
                        op0=mybir.AluOpType.mult, op1=mybir.AluOpType.add)

            def emit_v_bias(vt, half):
                if bv_sb is not None:
                    nc.tensor.matmul(
                        vt[:, half * 256:(half + 1) * 256],
                        lhsT=ones_sb, rhs=bv_sb,
                        start=False, stop=True, skip_group_check=True)

            def emit_v_evac(vt, sb0):
                # vt [s=128, (sb2 h4 d64)] -> v16[:, h, sb, 0:64]
                nc.vector.tensor_copy(
                    out=v16[:, :, sb0:sb0 + 2, 0:DK],
                    in_=vt[:].rearrange("p (s h d) -> p h s d", s=2, h=4))

            with tc.tile_pool(name="pa0", bufs=1, space="PSUM") as pa0:
                qk0 = [pa0.tile([128, 512], F32, name=f"qk0_{cc}")
                       for cc in range(4)]

                def sc0_mm(dc, sb):
                    for cc in range(4):
                        nc.tensor.matmul(
                            qk0[cc][:, sb * 128:(sb + 1) * 128],
                            lhsT=w16[:, dc, cc * 128:(cc + 1) * 128],
                            rhs=xT16[:, dc, sb * 128:(sb + 1) * 128],
                            start=(dc == 0 and sb == 0), stop=(dc == 7),
                            skip_group_check=True)

                # arrival-ordered emission: (w half A: dc0-3 | B: dc4-7),
                # x s-blocks land 0,1,2,3.  Warm-up filler between waves
                # keeps the PE ramp alive while DMA catches up.
                for dc in range(4):
                    sc0_mm(dc, 0)
                emit_warm(2)
                for dc in range(4):
                    sc0_mm(dc, 1)
                emit_warm(2)
                for dc in range(4, 8):
                    sc0_mm(dc, 0)
                for dc in range(4, 8):
                    sc0_mm(dc, 1)
                for dc in range(8):
                    sc0_mm(dc, 2)
                for dc in range(8):
                    sc0_mm(dc, 3)
                # ACT is idle here (no exps yet): split the 4 evacs across
                # DVE and ACT so the A0->A1 psum-bank handoff drains fast
                for cc in range(4):
                    emit_qk_evac(qk0[cc], cc, 0, on_act=(cc % 2 == 1))
                # dependency-free warms keep PE busy (and the ramp alive)
                # while the evacs drain and the A1 pool takes over the banks
                emit_warm(2)
            warm_ctx.__exit__(None, None, None)

            # =========== QK/exp machinery =================================
            st_ctx = tc.tile_pool(name="pb_st", bufs=1, space="PSUM")
            st_ps = st_ctx.__enter__()
            pt_ctx = tc.tile_pool(name="pb_pt", bufs=40)
            pt_pool = pt_ctx.__enter__()

            ptmap = {}      # (hh, hp, grp) -> P^T group tile

            def emit_qk_group(hh, grp):
                lo = grp * 8
                chunk = pairs[lo:lo + 8]
                sts = [st_ps.tile([128, 1024], F32,
                                  name=f"st{hp}_{hh}_{grp}", tag=f"st{hp}")
                       for hp in range(2)]
                # merge consecutive active rows at the same key block into
                # one wider matmul (same stationary K_j, moving N up to 512;
                # may not cross a psum bank)
                qk_runs = []
                for sl, (j, r) in enumerate(chunk):
                    if (qk_runs and qk_runs[-1][0] == j
                            and qk_runs[-1][1] + qk_runs[-1][3] == r
                            and qk_runs[-1][2] // 4 == sl // 4
                            and qk_runs[-1][3] < 4):
                        qk_runs[-1][3] += 1
                    else:
                        qk_runs.append([j, r, sl, 1])
                for j, r0, sl0, L in qk_runs:
                    for hp in range(2):
                        bp = hp * 64
                        nc.tensor.matmul(
                            sts[hp][:, sl0 * 128:(sl0 + L) * 128],
                            lhsT=k16[bp:bp + 64, hh, j * 128:(j + 1) * 128],
                            rhs=q16[bp:bp + 64, hh, r0 * 128:(r0 + L) * 128],
                            start=True, stop=True)
                for hp in range(2):
                    ptg = pt_pool.tile([128, 1024], F16,
                                       name=f"ptg_{hh}_{hp}_{grp}", tag="ptg")
                    ptmap[(hh, hp, grp)] = ptg
                    nc.scalar.activation(
                        out=ptg[:, 0:len(chunk) * 128],
                        in_=sts[hp][:, 0:len(chunk) * 128], func=EXP)

            # pending group queue in emission priority order: alternate hh so
            # both heads' P^T becomes available evenly.
            grp_order = []
            for g in range(n_grp):
                for hh in range(2):
                    grp_order.append((hh, g))
            g_next = 0          # next index into grp_order not yet emitted
            emitted = set()

            def emit_next_group_if(maxblock):
                nonlocal g_next
                if (g_next < len(grp_order)
                        and grp_need[grp_order[g_next][1]] <= maxblock):
                    hh, g = grp_order[g_next]
                    emit_qk_group(hh, g)
                    emitted.add((hh, g))
                    g_next += 1
                    return True
                return False

            # =========== phase A1: Q/K s-chunks 1..3, then V, with QK
            # groups interleaved.  All of Q/K is projected FIRST so every
            # attention group unlocks as early as possible (ACT is the
            # long pole); the V projection runs after as PE filler.
            with tc.tile_pool(name="pa1", bufs=2, space="PSUM") as pa1:
                for sc in range(1, 4):
                    done_block = sc * 4 - 1   # blocks < sc*4 are projected
                    for cc in range(4):
                        mm = pa1.tile([128, 512], F32)
                        for dc in range(8):
                            nc.tensor.matmul(
                                mm,
                                lhsT=w16[:, dc, cc * 128:(cc + 1) * 128],
                                rhs=xT16[:, dc, sc * 512:(sc + 1) * 512],
                                start=(dc == 0), stop=(dc == 7))
                        emit_qk_evac(mm, cc, sc)
                        emit_next_group_if(
                            done_block if cc < 3 else sc * 4 + 3)
                for p in range(8):
                    vt = pa1.tile([128, 512], F32)
                    for half in range(2):
                        sb = p * 2 + half
                        for dc in range(8):
                            nc.tensor.matmul(
                                vt[:, half * 256:(half + 1) * 256],
                                lhsT=xT16[:, dc, sb * 128:(sb + 1) * 128],
                                rhs=w16[:, dc, 512:768],
                                start=(dc == 0 and half == 0),
                                stop=(dc == 7) if bv_sb is None else False,
                                skip_group_check=True)
                        emit_v_bias(vt, half)
                    emit_v_evac(vt, p * 2)
                    emit_next_group_if(TR - 1)

            # =========== phase B: PV + remaining QK groups ================
            # O' accumulated per (head, half-of-rows) into a [128, 1024]
            # (2-bank) psum tile, rows 0..63 = O, row 64 = l.  First matmul
            # into each bank carries start=True (whole-bank has_written
            # clear); later matmuls overwrite-or-accumulate per element.
            # hp0's (1/l)-multiply is fused with the PSUM evacuation (same
            # base partition); hp1 copies to oT16 first (cross-base copy is
            # legal, elementwise multiply needs aligned bases).
            with tc.tile_pool(name="pb_o", bufs=2, space="PSUM") as o_ps, \
                 tc.tile_pool(name="pb_div", bufs=2) as div_pool, \
                 tc.tile_pool(name="pc_sb", bufs=12) as fo_sb:

                fin_count = {0: 0, 1: 0}

                def head_items(hh, hp, half):
                    """Yield (req_groups, pe_cycles, emit_fn) for one
                    (head, row-half) O' accumulation."""
                    h = 2 * hh + hp
                    bp = hp * 64
                    HS = S // 2
                    state = {}

                    def ensure_tile():
                        if "o" not in state:
                            state["o"] = o_ps.tile(
                                [128, 1024], F32,
                                name=f"o_{hh}_{hp}_{half}", tag="o")
                            state["started"] = set()
                        return state["o"]

                    def runs_chunk(j4):
                        t = ensure_tile()
                        for j in range(j4 * 4, j4 * 4 + 4):
                            for r0, ln, sp in runs_by_j[j]:
                                if r0 // 8 != half:
                                    continue
                                off = pk[(j, r0)]
                                ptg = ptmap[(hh, hp, off // 8)]
                                o8 = off % 8
                                c0 = (r0 - half * 8) * 128
                                bank = (r0 - half * 8) // 4
                                st = bank not in state["started"]
                                state["started"].add(bank)
                                nc.tensor.matmul(
                                    t[0:DK + 1, c0:c0 + ln * 128],
                                    lhsT=v16[:, h, j, :],
                                    rhs=ptg[:, o8 * 128:(o8 + ln) * 128],
                                    start=st, stop=sp,
                                    skip_group_check=True)

                    for j4 in range(4):
                        req, cyc = set(), 0
                        for j in range(j4 * 4, j4 * 4 + 4):
                            for r0, ln, sp in runs_by_j[j]:
                                if r0 // 8 == half:
                                    req.add((hh, pk[(j, r0)] // 8))
                                    cyc += ln * 128
                        if cyc:
                            yield (req, cyc, lambda j4=j4: runs_chunk(j4))

                    def finish_half():
                        t = ensure_tile()
                        for r in empty_rows:
                            if r // 8 != half:
                                continue
                            c0 = (r - half * 8) * 128
                            nc.vector.memset(t[0:DK, c0:c0 + 128], 0.0)
                            nc.vector.memset(t[DK:DK + 1, c0:c0 + 128], 1.0)
                        dst = oT16[bp:bp + 64, hh,
                                   half * HS:(half + 1) * HS]
                        linv = div_pool.tile(
                            [1, HS], F32, name=f"linv_{h}_{half}",
                            tag="linv")
                        lb = div_pool.tile(
                            [128, HS], F32, name=f"lb_{h}_{half}", tag="lb")
                        late = half == 1 and fin_count[1] >= 2
                        nc.vector.reciprocal(linv, t[DK:DK + 1, :])
                        nc.gpsimd.partition_broadcast(lb, linv)
                        if hp == 0:
                            if late:    # split: first 512 cols unlock the
                                        # next out-proj stripe early
                                nc.vector.tensor_mul(
                                    dst[:, 0:512], t[0:DK, 0:512],
                                    lb[0:64, 0:512])
                                nc.vector.tensor_mul(
                                    dst[:, 512:1024], t[0:DK, 512:1024],
                                    lb[0:64, 512:1024])
                            else:
                                nc.vector.tensor_mul(dst, t[0:DK, :],
                                                     lb[0:64, :])
                        else:
                            if late:    # ACT queue is drained by now
                                nc.scalar.copy(out=dst, in_=t[0:DK, :])
                            else:
                                nc.vector.tensor_copy(out=dst,
                                                      in_=t[0:DK, :])
                            if late:
                                nc.vector.tensor_mul(
                                    dst[:, 0:512], dst[:, 0:512],
                                    lb[bp:bp + 64, 0:512])
                                nc.vector.tensor_mul(
                                    dst[:, 512:1024], dst[:, 512:1024],
                                    lb[bp:bp + 64, 512:1024])
                            else:
                                nc.vector.tensor_mul(dst, dst,
                                                     lb[bp:bp + 64, :])
                        fin_count[half] += 1
                    allreq = {(hh, pk[(j, r0)] // 8)
                              for j in range(TR)
                              for r0, ln, sp in runs_by_j[j]
                              if r0 // 8 == half}
                    yield (allreq, 0, finish_half)

                # (head, half) units in pipeline order: all half-0 units
                # before half-1, so oT16 columns complete in order and the
                # output projection's first half unblocks first.
                pv_stream = []      # (req_groups, pe_cycles, emit_fn)
                for half in range(2):
                    for hh in range(2):
                        for hp in range(2):
                            pv_stream.extend(head_items(hh, hp, half))
                pv_i = 0
                c_next = 0

                def c_ready(sb):
                    return fin_count[sb // 8] >= 4

                # ---- output projection, sharing the "o" psum tag: each
                # C-tile is [128, 1024] = 2 banks, one per e-half.  sb 0-7
                # only need the half-0 finishes and act as PE filler inside
                # the ACT-bound group window; their evacs stay off ACT so
                # the exp pipeline is never delayed.
                c_pool_cycle = [0]
                c_pre = {}      # sb -> fo tile with the hh0 half done

                def emit_c_pre(sb, tag):
                    # hh0's half-1 O is finished before hh1's: start the
                    # out-proj accumulation for a late stripe early in an
                    # idle S^T bank; the hh1 matmuls complete it later.
                    fo = st_ps.tile([128, 1024], F32, name=f"c_{sb}",
                                    tag=tag)
                    for e in range(2):
                        nc.tensor.matmul(
                            fo[:, e * 512:(e + 1) * 512],
                            lhsT=oT16[:, 0, sb * 128:(sb + 1) * 128],
                            rhs=wo16[:, 0, e * 512:(e + 1) * 512],
                            start=True, stop=False, skip_group_check=True)
                    c_pre[sb] = fo

                def emit_c_sb(sb):
                    act_ok = sb >= 8    # late sbs: exps done, ACT helps
                    if sb in c_pre:
                        fo = c_pre.pop(sb)
                        hhs = [1]
                    else:
                        if g_next >= len(grp_order):
                            # groups done: idle S^T banks double the number
                            # of out-proj tiles in flight
                            srcs = [(o_ps, "o"), (st_ps, "st0"),
                                    (st_ps, "st1")]
                            pool, tag = srcs[c_pool_cycle[0] % 3]
                            c_pool_cycle[0] += 1
                        else:
                            pool, tag = o_ps, "o"
                        fo = pool.tile([128, 1024], F32, name=f"c_{sb}",
                                       tag=tag)
                        hhs = [0, 1]
                    st = fo_sb.tile([128, D], F16)
                    for e in range(2):
                        for hh in hhs:
                            nc.tensor.matmul(
                                fo[:, e * 512:(e + 1) * 512],
                                lhsT=oT16[:, hh, sb * 128:(sb + 1) * 128],
                                rhs=wo16[:, hh, e * 512:(e + 1) * 512],
                                start=(hh == 0), stop=(hh == 1),
                                skip_group_check=True)
                    if act_ok:
                        nc.vector.tensor_copy(
                            out=st[:, 0:512], in_=fo[:, 0:512])
                        nc.scalar.copy(
                            out=st[:, 512:1024], in_=fo[:, 512:1024])
                    else:
                        nc.vector.tensor_copy(out=st, in_=fo)
                    nc.sync.dma_start(
                        out=out[sb * 128:(sb + 1) * 128, :], in_=st)

                fill_total = 0

                def emit_fillers(target):
                    nonlocal pv_i, c_next, fill_total
                    while fill_total < target:
                        if pv_i < len(pv_stream):
                            req, cyc, fn = pv_stream[pv_i]
                            if req <= emitted:
                                fn()
                                fill_total += cyc
                                pv_i += 1
                                continue
                        if c_next < TR and c_ready(c_next):
                            emit_c_sb(c_next)
                            c_next += 1
                            fill_total += 2048
                            continue
                        break

                # steady state: pace QK groups with PV/out-proj filler so
                # the in-order PE queue alternates (QK paces ACT, the
                # filler keeps PE busy while ACT chews the exps).  The
                # filler target is cumulative, so emission blocked in one
                # round (P^T group not yet live) is caught up in the next.
                grp_done = 0
                while g_next < len(grp_order):
                    emit_fillers((grp_done + 1) * 2750)
                    emit_next_group_if(TR - 1)
                    grp_done += 1
                while pv_i < len(pv_stream):
                    req, cyc, fn = pv_stream[pv_i]
                    assert req <= emitted
                    fn()
                    pv_i += 1
                    if fin_count[1] == 2 and not c_pre and c_next <= 8:
                        emit_c_pre(8, "st0")
                        emit_c_pre(9, "st1")
                    while c_next < TR and c_ready(c_next):
                        emit_c_sb(c_next)
                        c_next += 1
                while c_next < TR:
                    emit_c_sb(c_next)
                    c_next += 1

            st_ctx.__exit__(None, None, None)
            pt_ctx.__exit__(None, None, None)
    nc.finalize()
    return nc


def _shard_inputs(x, w_qkv, b_qkv, w_o, scale):
    in_maps = []
    xts = [np.ascontiguousarray(x[b].T, dtype=np.float16) for b in range(B)]
    for c in range(NCORES):
        b, h0 = c // 4, (c % 4) * HL
        q = slice(h0 * DK, h0 * DK + DL)
        k = slice(D + h0 * DK, D + h0 * DK + DL)
        v = slice(2 * D + h0 * DK, 2 * D + h0 * DK + DL)
        wslice = np.concatenate([w_qkv[:, q], w_qkv[:, k], w_qkv[:, v]], axis=1)
        beff = np.concatenate([b_qkv[q] * scale, b_qkv[k], b_qkv[v]])
        im = {
            "xt": xts[b],
            "wqkv": np.ascontiguousarray(wslice, dtype=np.float16),
            "bias": np.ascontiguousarray(beff, dtype=np.float32),
            "wo": np.ascontiguousarray(w_o[h0 * DK:h0 * DK + DL, :],
                                       dtype=np.float16),
        }
        if np.any(b_qkv[v] != 0):
            im["bv"] = np.ascontiguousarray(
                b_qkv[v].reshape(1, DL), dtype=np.float16)
        in_maps.append(im)
    return in_maps


def kernel(x, w_qkv, b_qkv, w_o, b_o, tau, block_sparse_mask, _trace=False,
           **_run_kwargs):
    x = np.asarray(x, dtype=np.float32)
    w_qkv = np.asarray(w_qkv, dtype=np.float32)
    b_qkv = np.asarray(b_qkv, dtype=np.float32)
    w_o = np.asarray(w_o, dtype=np.float32)
    b_o = np.asarray(b_o, dtype=np.float32)
    mask = np.asarray(block_sparse_mask).astype(np.int64)
    scale = float(np.asarray(tau)) / math.sqrt(DK)

    vb = b_qkv[2 * D:]
    vbias16 = vb.astype(np.float16) if np.any(vb != 0) else None
    nc = _build_program(mask, scale, vbias16)
    in_maps = _shard_inputs(x, w_qkv, b_qkv, w_o, scale)
    res = run_bass_kernel_spmd(nc, in_maps, core_ids=list(range(NCORES)),
                               trace=_trace, **_run_kwargs)
    outs = [r["out"].astype(np.float32) for r in res.results]
    full = np.stack([
        outs[0] + outs[1] + outs[2] + outs[3] + b_o,
        outs[4] + outs[5] + outs[6] + outs[7] + b_o,
    ]).astype(np.float32)
    if _trace:
        kernel.last_result = res
    return full


# revision 68
# speedup vs baseline: 1.0107x; 1.0048x over previous
"""Block-sparse multi-head attention (QKV proj + sparse flash + out proj)
for Trainium2, sharded over 8 NeuronCores as batch x head-group.

Layout of the per-core program (SPMD: identical program on all 8 cores,
all sharding done host-side via per-core input slices):

  core c: batch b = c // 4, heads h0 = (c % 4) * 4 .. h0 + 4.
  inputs : xt    [1024, 2048] f16   x[b] TRANSPOSED on the host (d-major),
                                    so the device never transposes x
           wqkv  [1024, 768]  f16   w_qkv columns for the core's 4 heads,
                                    re-packed as [q(256) | k(256) | v(256)]
           bias  [768]        f32   matching b_qkv slice (q part pre-scaled
                                    by tau/sqrt(dk))
           wo    [256, 1024]  f16   w_o rows for the core's heads
  output : out   [2048, 1024] f16   partial output projection (host sums the
                                    4 head-group partials per batch, + b_o)

The block mask (16x16, shared by every head/batch) is known at program
build time, so the kernel is specialized to it: only active (qblock,
kblock) pairs get score/exp/PV work.  Softmax is computed without the
running-max pass: scores are ~N(0,1) for this problem family, so exp()
stays comfortably inside fp32/fp16 range, and softmax(S) is
mathematically identical with or without the max shift.

Everything flows transposed (dk-major) so no transpose is ever needed:
  Q^T,K^T  from x^T via c-major projection (contraction d on partitions)
  V        via s-major projection (same xT/w tiles, swapped operands)
  S^T[k,q] = K_j @ Q_r^T        (lhsT = K dk-major, rhs = Q dk-major)
  P^T      = exp(S^T)           (ACT, packed by active pair -> fp16)
  O'^T     = sum_j V'_j^T @ P^T (V' carries a ones column so one PSUM row
                                 accumulates l = sum P; the ones column
                                 sits below O for head-pair half 0 and
                                 above it for half 1 so the 1/l multiply
                                 operands share a base partition)
  O^T      = O'^T * (1/l)       (gpsimd partition-broadcast of 1/l,
                                 multiply fused with the PSUM evacuation)
  out      = O^T.T @ Wo         (lhsT = O^T blocks, rhs = Wo rows)

PSUM accumulation never uses zero-init matmuls: the first matmul into
each PSUM bank of an accumulation group is issued with start=True (which
clears the whole bank's has_written bits); later matmuls overwrite where
the bit is clear and accumulate where it is set, which is exactly the
per-element semantics needed.

The PE p-state ramp (0.65/1.2 GHz for the first ~3us of busy time) is
burned with dependency-free warm-up matmuls while the first DMAs land,
so real work starts at full clock.
"""

import math
import sys

import numpy as np

for _p in ("/opt/trn_rl_repo", "/root/.axon_site/_ro/trn_rl_repo"):
    if _p not in sys.path:
        sys.path.insert(0, _p)

import concourse.bass as bass
import concourse.mybir as mybir
import concourse.tile as tile
from concourse import bacc
from concourse.bass_utils import run_bass_kernel_spmd

H = 16      # total heads
DK = 64     # head dim
BS = 128    # block size
S = 2048    # sequence length
D = 1024    # model dim
B = 2       # batch
NCORES = 8
HL = 4      # heads per core
DL = HL * DK          # 256 local qkv width
TR = S // BS          # 16 blocks

F32 = mybir.dt.float32
F16 = mybir.dt.float16
EXP = mybir.ActivationFunctionType.Exp

N_WARM = 10           # warm-up matmuls (512 cols each) to cover the ramp


def _build_program(mask, scale, vbias16=None):
    """mask: [16,16] 0/1 array (build-time constant). scale: tau/sqrt(dk).
    vbias16: fp16 [256] v-part bias, or None when it is all zeros."""
    nc = bacc.Bacc("TRN2", target_bir_lowering=False)

    xt = nc.dram_tensor("xt", [D, S], F16, kind="ExternalInput")
    wqkv = nc.dram_tensor("wqkv", [D, 3 * DL], F16, kind="ExternalInput")
    bias = nc.dram_tensor("bias", [3 * DL], F32, kind="ExternalInput")
    wo = nc.dram_tensor("wo", [DL, D], F16, kind="ExternalInput")
    out = nc.dram_tensor("out", [S, D], F16, kind="ExternalOutput")
    bv = (nc.dram_tensor("bv", [1, DL], F16, kind="ExternalInput")
          if vbias16 is not None else None)

    # ---- build-time sparsity bookkeeping (mask shared by all heads) ----
    act_r = [[r for r in range(TR) if mask[r][j]] for j in range(TR)]
    first_j, last_j = {}, {}
    for r in range(TR):
        js = [j for j in range(TR) if mask[r][j]]
        if js:
            first_j[r], last_j[r] = js[0], js[-1]
    empty_rows = [r for r in range(TR) if r not in first_j]
    # Packed S^T/P^T slot order: all row-half-0 (r<8) pairs first, then
    # half 1 — so the first O'-half (and with it the first half of the
    # output projection) completes before the second half's exps finish.
    # Within a half, pairs are bucketed by the highest q/k block they
    # touch (max(j,r)//4), so the first groups only need the first
    # projection s-chunk and the QK->exp pipeline starts as early as
    # possible.  Bucket boundaries coincide with the r//4 PV-run limit,
    # so PV run merging is unaffected.
    pk = {}          # (j, r) -> packed slot index in P^T
    gidx = 0
    for half in range(2):
        for need in range(4):
            for j in range(TR):
                for r in act_r[j]:
                    if r // 8 == half and max(j, r) // 4 == need:
                        pk[(j, r)] = gidx
                        gidx += 1
    nact = gidx

    # runs of consecutive active rows at one key block, uniform stop flag.
    # Runs may not cross an O' psum bank (r//4) nor a P^T group tile
    # (packed slot // 8) boundary.
    def pv_runs(j):
        runs = []
        for r in act_r[j]:
            sp = last_j[r] == j
            if (runs and runs[-1][0] + runs[-1][1] == r
                    and runs[-1][2] == sp
                    and runs[-1][1] < 4
                    and runs[-1][0] // 4 == r // 4
                    and pk[(j, runs[-1][0])] // 8 == pk[(j, r)] // 8):
                runs[-1][1] += 1
            else:
                runs.append([r, 1, sp])
        return runs

    runs_by_j = {j: pv_runs(j) for j in range(TR)}
    pairs = sorted(pk, key=lambda jr: pk[jr])
    n_grp = (nact + 7) // 8
    # group g touches blocks up to grp_need[g]; its QK can be emitted once
    # the projection has produced q/k for all blocks <= that.
    grp_need = [max(max(j, r) for j, r in pairs[g * 8:(g + 1) * 8])
                for g in range(n_grp)]

    with tile.TileContext(nc) as tc:
        with tc.tile_pool(name="persist", bufs=1) as persist:
            bias_sb = persist.tile([128, 6], F32)
            w16 = persist.tile([128, 8, 3 * DL], F16)
            wo16 = persist.tile([128, 2, D], F16)
            xT16 = persist.tile([128, 8, S], F16)
            q16 = persist.tile([128, 2, S], F16)
            k16 = persist.tile([128, 2, S], F16)
            # V' = [V | ones]: PSUM row 64 of each O' tile accumulates
            # l = sum P via the ones column.
            v16 = persist.tile([128, HL, TR, DK + 1], F16)
            oT16 = persist.tile([128, 2, S], F16)
            warm = persist.tile([128, 512], F16)
            bv_sb = persist.tile([1, DL], F16) if bv is not None else None
            ones_sb = persist.tile([1, 128], F16) if bv is not None else None

            xt_r = xt[:].rearrange("(dc p) s -> p dc s", p=128)
            wqkv_r = wqkv[:].rearrange("(dc p) c -> p dc c", p=128)
            wo_r = wo[:].rearrange("(cc p) e -> p cc e", p=128)

            # ---- DMA stream (serial HWDGE/DMA engines; order == priority)
            nc.sync.dma_start(out=xT16[:, :, 0:128], in_=xt_r[:, :, 0:128])
            nc.sync.dma_start(out=w16[:, 0:4, :], in_=wqkv_r[:, 0:4, :])
            nc.sync.dma_start(out=xT16[:, :, 128:256], in_=xt_r[:, :, 128:256])
            nc.sync.dma_start(out=w16[:, 4:8, :], in_=wqkv_r[:, 4:8, :])
            nc.sync.dma_start(out=xT16[:, :, 256:384], in_=xt_r[:, :, 256:384])
            nc.sync.dma_start(out=xT16[:, :, 384:512], in_=xt_r[:, :, 384:512])
            nc.sync.dma_start(out=bias_sb,
                              in_=bias[:].rearrange("(g p) -> p g", p=128))
            for sc in range(1, 4):
                nc.sync.dma_start(out=xT16[:, :, sc * 512:(sc + 1) * 512],
                                  in_=xt_r[:, :, sc * 512:(sc + 1) * 512])
            nc.sync.dma_start(out=wo16, in_=wo_r)
            if bv is not None:
                nc.sync.dma_start(out=bv_sb, in_=bv[:])

            nc.vector.memset(warm, 0.0)
            for h in range(HL):
                nc.vector.memset(v16[:, h, :, DK:DK + 1], 1.0)
            if ones_sb is not None:
                nc.vector.memset(ones_sb, 1.0)
            # dummy exp so the ACT table load runs at t~0, off the first
            # QK group's critical path
            dummy = persist.tile([1, 8], F16)
            nc.scalar.activation(out=dummy, in_=warm[0:1, 0:8], func=EXP)

            # ---- PE warm-up: dependency-free matmuls burn the p-state ramp
            # while the first x/w DMAs land.
            warm_ctx = tc.tile_pool(name="warm_ps", bufs=1, space="PSUM")
            warm_ps = warm_ctx.__enter__()
            wtile = warm_ps.tile([128, 512], F32)

            def emit_warm(n):
                for _ in range(n):
                    nc.tensor.matmul(wtile, lhsT=warm[:, 0:128], rhs=warm,
                                     start=True, stop=True)
            emit_warm(N_WARM)

            # =========== phase A0: s-chunk 0, dc-outer (DMA-paced) ========
            # 6 concurrently-accumulating PSUM banks: 4 Q/K c-chunks of
            # [c=128, s=512] and 2 V tiles of [s=128 x 2, c=256 halves];
            # matmuls are emitted at (dc, sb) granularity in DMA arrival
            # order (w comes in two dc-halves, x in four s-blocks).
            def emit_qk_evac(mm, cc, sc, on_act=False):
                if cc < 2:
                    dst, sc_imm = q16[:, cc, sc * 512:(sc + 1) * 512], scale
                else:
                    dst, sc_imm = k16[:, cc - 2, sc * 512:(sc + 1) * 512], 1.0
                if on_act:      # ACT: out = Ident(in * scale + bias)
                    nc.scalar.activation(
                        out=dst, in_=mm,
                        func=mybir.ActivationFunctionType.Identity,
                        bias=bias_sb[:, cc:cc + 1], scale=sc_imm)
                else:
                    nc.vector.tensor_scalar(
                        out=dst, in0=mm, scalar1=sc_imm,
                        scalar2=bias_sb[:, cc:cc + 1],
                        op0=mybir.AluOpType.mult, op1=mybir.AluOpType.add)

            def emit_v_bias(vt, half):
                if bv_sb is not None:
                    nc.tensor.matmul(
                        vt[:, half * 256:(half + 1) * 256],
                        lhsT=ones_sb, rhs=bv_sb,
                        start=False, stop=True, skip_group_check=True)

            def emit_v_evac(vt, sb0):
                # vt [s=128, (sb2 h4 d64)] -> v16[:, h, sb, 0:64]
                nc.vector.tensor_copy(
                    out=v16[:, :, sb0:sb0 + 2, 0:DK],
                    in_=vt[:].rearrange("p (s h d) -> p h s d", s=2, h=4))

            with tc.tile_pool(name="pa0", bufs=1, space="PSUM") as pa0:
                qk0 = [pa0.tile([128, 512], F32, name=f"qk0_{cc}")
                       for cc in range(4)]

                def sc0_mm(dc, sb):
                    for cc in range(4):
                        nc.tensor.matmul(
                            qk0[cc][:, sb * 128:(sb + 1) * 128],
                            lhsT=w16[:, dc, cc * 128:(cc + 1) * 128],
                            rhs=xT16[:, dc, sb * 128:(sb + 1) * 128],
                            start=(dc == 0 and sb == 0), stop=(dc == 7),
                            skip_group_check=True)

                # arrival-ordered emission: (w half A: dc0-3 | B: dc4-7),
                # x s-blocks land 0,1,2,3.  Warm-up filler between waves
                # keeps the PE ramp alive while DMA catches up.
                for dc in range(4):
                    sc0_mm(dc, 0)
                emit_warm(2)
                for dc in range(4):
                    sc0_mm(dc, 1)
                emit_warm(2)
                for dc in range(4, 8):
                    sc0_mm(dc, 0)
                for dc in range(4, 8):
                    sc0_mm(dc, 1)
                for dc in range(8):
                    sc0_mm(dc, 2)
                for dc in range(8):
                    sc0_mm(dc, 3)
                # ACT is idle here (no exps yet): split the 4 evacs across
                # DVE and ACT so the A0->A1 psum-bank handoff drains fast
                for cc in range(4):
                    emit_qk_evac(qk0[cc], cc, 0, on_act=(cc % 2 == 1))
                # dependency-free warms keep PE busy (and the ramp alive)
                # while the evacs drain and the A1 pool takes over the banks
                emit_warm(2)
            warm_ctx.__exit__(None, None, None)

            # =========== QK/exp machinery =================================
            st_ctx = tc.tile_pool(name="pb_st", bufs=1, space="PSUM")
            st_ps = st_ctx.__enter__()
            pt_ctx = tc.tile_pool(name="pb_pt", bufs=40)
            pt_pool = pt_ctx.__enter__()

            ptmap = {}      # (hh, hp, grp) -> P^T group tile

            def emit_qk_group(hh, grp):
                lo = grp * 8
                chunk = pairs[lo:lo + 8]
                sts = [st_ps.tile([128, 1024], F32,
                                  name=f"st{hp}_{hh}_{grp}", tag=f"st{hp}")
                       for hp in range(2)]
                # merge consecutive active rows at the same key block into
                # one wider matmul (same stationary K_j, moving N up to 512;
                # may not cross a psum bank)
                qk_runs = []
                for sl, (j, r) in enumerate(chunk):
                    if (qk_runs and qk_runs[-1][0] == j
                            and qk_runs[-1][1] + qk_runs[-1][3] == r
                            and qk_runs[-1][2] // 4 == sl // 4
                            and qk_runs[-1][3] < 4):
                        qk_runs[-1][3] += 1
                    else:
                        qk_runs.append([j, r, sl, 1])
                for j, r0, sl0, L in qk_runs:
                    for hp in range(2):
                        bp = hp * 64
                        nc.tensor.matmul(
                            sts[hp][:, sl0 * 128:(sl0 + L) * 128],
                            lhsT=k16[bp:bp + 64, hh, j * 128:(j + 1) * 128],
                            rhs=q16[bp:bp + 64, hh, r0 * 128:(r0 + L) * 128],
                            start=True, stop=True)
                for hp in range(2):
                    ptg = pt_pool.tile([128, 1024], F16,
                                       name=f"ptg_{hh}_{hp}_{grp}", tag="ptg")
                    ptmap[(hh, hp, grp)] = ptg
                    nc.scalar.activation(
                        out=ptg[:, 0:len(chunk) * 128],
                        in_=sts[hp][:, 0:len(chunk) * 128], func=EXP)

            # pending group queue in emission priority order: alternate hh so
            # both heads' P^T becomes available evenly.
            grp_order = []
            for g in range(n_grp):
                for hh in range(2):
                    grp_order.append((hh, g))
            g_next = 0          # next index into grp_order not yet emitted
            emitted = set()

            def emit_next_group_if(maxblock):
                nonlocal g_next
                if (g_next < len(grp_order)
                        and grp_need[grp_order[g_next][1]] <= maxblock):
                    hh, g = grp_order[g_next]
                    emit_qk_group(hh, g)
                    emitted.add((hh, g))
                    g_next += 1
                    return True
                return False

            # =========== phase A1: Q/K s-chunks 1..3, then V, with QK
            # groups interleaved.  All of Q/K is projected FIRST so every
            # attention group unlocks as early as possible (ACT is the
            # long pole); the V projection runs after as PE filler.
            with tc.tile_pool(name="pa1", bufs=2, space="PSUM") as pa1:
                for sc in range(1, 4):
                    done_block = sc * 4 - 1   # blocks < sc*4 are projected
                    for cc in range(4):
                        mm = pa1.tile([128, 512], F32)
                        for dc in range(8):
                            nc.tensor.matmul(
                                mm,
                                lhsT=w16[:, dc, cc * 128:(cc + 1) * 128],
                                rhs=xT16[:, dc, sc * 512:(sc + 1) * 512],
                                start=(dc == 0), stop=(dc == 7))
                        emit_qk_evac(mm, cc, sc)
                        emit_next_group_if(
                            done_block if cc < 3 else sc * 4 + 3)
                for p in range(8):
                    vt = pa1.tile([128, 512], F32)
                    for half in range(2):
                        sb = p * 2 + half
                        for dc in range(8):
                            nc.tensor.matmul(
                                vt[:, half * 256:(half + 1) * 256],
                                lhsT=xT16[:, dc, sb * 128:(sb + 1) * 128],
                                rhs=w16[:, dc, 512:768],
                                start=(dc == 0 and half == 0),
                                stop=(dc == 7) if bv_sb is None else False,
                                skip_group_check=True)
                        emit_v_bias(vt, half)
                    emit_v_evac(vt, p * 2)
                    emit_next_group_if(TR - 1)

            # =========== phase B: PV + remaining QK groups ================
            # O' accumulated per (head, half-of-rows) into a [128, 1024]
            # (2-bank) psum tile, rows 0..63 = O, row 64 = l.  First matmul
            # into each bank carries start=True (whole-bank has_written
            # clear); later matmuls overwrite-or-accumulate per element.
            # hp0's (1/l)-multiply is fused with the PSUM evacuation (same
            # base partition); hp1 copies to oT16 first (cross-base copy is
            # legal, elementwise multiply needs aligned bases).
            with tc.tile_pool(name="pb_o", bufs=2, space="PSUM") as o_ps, \
                 tc.tile_pool(name="pb_div", bufs=2) as div_pool, \
                 tc.tile_pool(name="pc_sb", bufs=12) as fo_sb:

                fin_count = {0: 0, 1: 0}

                def head_items(hh, hp, half):
                    """Yield (req_groups, pe_cycles, emit_fn) for one
                    (head, row-half) O' accumulation."""
                    h = 2 * hh + hp
                    bp = hp * 64
                    HS = S // 2
                    state = {}

                    def ensure_tile():
                        if "o" not in state:
                            state["o"] = o_ps.tile(
                                [128, 1024], F32,
                                name=f"o_{hh}_{hp}_{half}", tag="o")
                            state["started"] = set()
                        return state["o"]

                    def runs_chunk(j4):
                        t = ensure_tile()
                        for j in range(j4 * 4, j4 * 4 + 4):
                            for r0, ln, sp in runs_by_j[j]:
                                if r0 // 8 != half:
                                    continue
                                off = pk[(j, r0)]
                                ptg = ptmap[(hh, hp, off // 8)]
                                o8 = off % 8
                                c0 = (r0 - half * 8) * 128
                                bank = (r0 - half * 8) // 4
                                st = bank not in state["started"]
                                state["started"].add(bank)
                                nc.tensor.matmul(
                                    t[0:DK + 1, c0:c0 + ln * 128],
                                    lhsT=v16[:, h, j, :],
                                    rhs=ptg[:, o8 * 128:(o8 + ln) * 128],
                                    start=st, stop=sp,
                                    skip_group_check=True)

                    for j4 in range(4):
                        req, cyc = set(), 0
                        for j in range(j4 * 4, j4 * 4 + 4):
                            for r0, ln, sp in runs_by_j[j]:
                                if r0 // 8 == half:
                                    req.add((hh, pk[(j, r0)] // 8))
                                    cyc += ln * 128
                        if cyc:
                            yield (req, cyc, lambda j4=j4: runs_chunk(j4))

                    def finish_half():
                        t = ensure_tile()
                        for r in empty_rows:
                            if r // 8 != half:
                                continue
                            c0 = (r - half * 8) * 128
                            nc.vector.memset(t[0:DK, c0:c0 + 128], 0.0)
                            nc.vector.memset(t[DK:DK + 1, c0:c0 + 128], 1.0)
                        dst = oT16[bp:bp + 64, hh,
                                   half * HS:(half + 1) * HS]
                        linv = div_pool.tile(
                            [1, HS], F32, name=f"linv_{h}_{half}",
                            tag="linv")
                        lb = div_pool.tile(
                            [128, HS], F32, name=f"lb_{h}_{half}", tag="lb")
                        late = half == 1 and fin_count[1] >= 2
                        if late:
                            # the whole chain runs per 512-column bank so
                            # the last out-proj stripes unlock earlier; the
                            # ACT queue is drained by now, so hp1's O-copy
                            # runs there in parallel with DVE
                            for b in range(2):
                                cs = slice(b * 512, (b + 1) * 512)
                                nc.vector.reciprocal(
                                    linv[:, cs], t[DK:DK + 1, cs])
                                nc.gpsimd.partition_broadcast(
                                    lb[:, cs], linv[:, cs])
                                if hp == 0:
                                    nc.vector.tensor_mul(
                                        dst[:, cs], t[0:DK, cs],
                                        lb[0:64, cs])
                                else:
                                    nc.scalar.copy(out=dst[:, cs],
                                                   in_=t[0:DK, cs])
                                    nc.vector.tensor_mul(
                                        dst[:, cs], dst[:, cs],
                                        lb[bp:bp + 64, cs])
                        else:
                            nc.vector.reciprocal(linv, t[DK:DK + 1, :])
                            nc.gpsimd.partition_broadcast(lb, linv)
                            if hp == 0:
                                nc.vector.tensor_mul(dst, t[0:DK, :],
                                                     lb[0:64, :])
                            else:
                                nc.vector.tensor_copy(out=dst,
                                                      in_=t[0:DK, :])
                                nc.vector.tensor_mul(dst, dst,
                                                     lb[bp:bp + 64, :])
                        fin_count[half] += 1
                    allreq = {(hh, pk[(j, r0)] // 8)
                              for j in range(TR)
                              for r0, ln, sp in runs_by_j[j]
                              if r0 // 8 == half}
                    yield (allreq, 0, finish_half)

                # (head, half) units in pipeline order: all half-0 units
                # before half-1, so oT16 columns complete in order and the
                # output projection's first half unblocks first.
                pv_stream = []      # (req_groups, pe_cycles, emit_fn)
                for half in range(2):
                    for hh in range(2):
                        for hp in range(2):
                            pv_stream.extend(head_items(hh, hp, half))
                pv_i = 0
                c_next = 0

                def c_ready(sb):
                    return fin_count[sb // 8] >= 4

                # ---- output projection, sharing the "o" psum tag: each
                # C-tile is [128, 1024] = 2 banks, one per e-half.  sb 0-7
                # only need the half-0 finishes and act as PE filler inside
                # the ACT-bound group window; their evacs stay off ACT so
                # the exp pipeline is never delayed.
                c_pool_cycle = [0]
                c_pre = {}      # sb -> fo tile with the hh0 half done

                def emit_c_pre(sb, tag):
                    # hh0's half-1 O is finished before hh1's: start the
                    # out-proj accumulation for a late stripe early in an
                    # idle S^T bank; the hh1 matmuls complete it later.
                    fo = st_ps.tile([128, 1024], F32, name=f"c_{sb}",
                                    tag=tag)
                    for e in range(2):
                        nc.tensor.matmul(
                            fo[:, e * 512:(e + 1) * 512],
                            lhsT=oT16[:, 0, sb * 128:(sb + 1) * 128],
                            rhs=wo16[:, 0, e * 512:(e + 1) * 512],
                            start=True, stop=False, skip_group_check=True)
                    c_pre[sb] = fo

                def emit_c_sb(sb):
                    act_ok = sb >= 8    # late sbs: exps done, ACT helps
                    if sb in c_pre:
                        fo = c_pre.pop(sb)
                        hhs = [1]
                    else:
                        if g_next >= len(grp_order):
                            # groups done: idle S^T banks double the number
                            # of out-proj tiles in flight
                            srcs = [(o_ps, "o"), (st_ps, "st0"),
                                    (st_ps, "st1")]
                            pool, tag = srcs[c_pool_cycle[0] % 3]
                            c_pool_cycle[0] += 1
                        else:
                            pool, tag = o_ps, "o"
                        fo = pool.tile([128, 1024], F32, name=f"c_{sb}",
                                       tag=tag)
                        hhs = [0, 1]
                    st = fo_sb.tile([128, D], F16)
                    for e in range(2):
                        for hh in hhs:
                            nc.tensor.matmul(
                                fo[:, e * 512:(e + 1) * 512],
                                lhsT=oT16[:, hh, sb * 128:(sb + 1) * 128],
                                rhs=wo16[:, hh, e * 512:(e + 1) * 512],
                                start=(hh == 0), stop=(hh == 1),
                                skip_group_check=True)
                    if act_ok:
                        nc.vector.tensor_copy(
                            out=st[:, 0:512], in_=fo[:, 0:512])
                        nc.scalar.copy(
                            out=st[:, 512:1024], in_=fo[:, 512:1024])
                    else:
                        nc.vector.tensor_copy(out=st, in_=fo)
                    nc.sync.dma_start(
                        out=out[sb * 128:(sb + 1) * 128, :], in_=st)

                fill_total = 0

                def emit_fillers(target):
                    nonlocal pv_i, c_next, fill_total
                    while fill_total < target:
                        if pv_i < len(pv_stream):
                            req, cyc, fn = pv_stream[pv_i]
                            if req <= emitted:
                                fn()
                                fill_total += cyc
                                pv_i += 1
                                continue
                        if c_next < TR and c_ready(c_next):
                            emit_c_sb(c_next)
                            c_next += 1
                            fill_total += 2048
                            continue
                        break

                # steady state: pace QK groups with PV/out-proj filler so
                # the in-order PE queue alternates (QK paces ACT, the
                # filler keeps PE busy while ACT chews the exps).  The
                # filler target is cumulative, so emission blocked in one
                # round (P^T group not yet live) is caught up in the next.
                grp_done = 0
                while g_next < len(grp_order):
                    emit_fillers((grp_done + 1) * 2750)
                    emit_next_group_if(TR - 1)
                    grp_done += 1
                while pv_i < len(pv_stream):
                    req, cyc, fn = pv_stream[pv_i]
                    assert req <= emitted
                    fn()
                    pv_i += 1
                    if fin_count[1] == 2 and not c_pre and c_next <= 8:
                        emit_c_pre(8, "st0")
                        emit_c_pre(9, "st1")
                    while c_next < TR and c_ready(c_next):
                        emit_c_sb(c_next)
                        c_next += 1
                while c_next < TR:
                    emit_c_sb(c_next)
                    c_next += 1

            st_ctx.__exit__(None, None, None)
            pt_ctx.__exit__(None, None, None)
    nc.finalize()
    return nc


def _shard_inputs(x, w_qkv, b_qkv, w_o, scale):
    in_maps = []
    xts = [np.ascontiguousarray(x[b].T, dtype=np.float16) for b in range(B)]
    for c in range(NCORES):
        b, h0 = c // 4, (c % 4) * HL
        q = slice(h0 * DK, h0 * DK + DL)
        k = slice(D + h0 * DK, D + h0 * DK + DL)
        v = slice(2 * D + h0 * DK, 2 * D + h0 * DK + DL)
        wslice = np.concatenate([w_qkv[:, q], w_qkv[:, k], w_qkv[:, v]], axis=1)
        beff = np.concatenate([b_qkv[q] * scale, b_qkv[k], b_qkv[v]])
        im = {
            "xt": xts[b],
            "wqkv": np.ascontiguousarray(wslice, dtype=np.float16),
            "bias": np.ascontiguousarray(beff, dtype=np.float32),
            "wo": np.ascontiguousarray(w_o[h0 * DK:h0 * DK + DL, :],
                                       dtype=np.float16),
        }
        if np.any(b_qkv[v] != 0):
            im["bv"] = np.ascontiguousarray(
                b_qkv[v].reshape(1, DL), dtype=np.float16)
        in_maps.append(im)
    return in_maps


def kernel(x, w_qkv, b_qkv, w_o, b_o, tau, block_sparse_mask, _trace=False,
           **_run_kwargs):
    x = np.asarray(x, dtype=np.float32)
    w_qkv = np.asarray(w_qkv, dtype=np.float32)
    b_qkv = np.asarray(b_qkv, dtype=np.float32)
    w_o = np.asarray(w_o, dtype=np.float32)
    b_o = np.asarray(b_o, dtype=np.float32)
    mask = np.asarray(block_sparse_mask).astype(np.int64)
    scale = float(np.asarray(tau)) / math.sqrt(DK)

    vb = b_qkv[2 * D:]
    vbias16 = vb.astype(np.float16) if np.any(vb != 0) else None
    nc = _build_program(mask, scale, vbias16)
    in_maps = _shard_inputs(x, w_qkv, b_qkv, w_o, scale)
    res = run_bass_kernel_spmd(nc, in_maps, core_ids=list(range(NCORES)),
                               trace=_trace, **_run_kwargs)
    outs = [r["out"].astype(np.float32) for r in res.results]
    full = np.stack([
        outs[0] + outs[1] + outs[2] + outs[3] + b_o,
        outs[4] + outs[5] + outs[6] + outs[7] + b_o,
    ]).astype(np.float32)
    if _trace:
        kernel.last_result = res
    return full
